# revision 52
# baseline (speedup 1.0000x reference)
"""Multi-head attention (B=2, S=2048, D=1024, H=16) on 8 NeuronCores.

Sharding: tensor-parallel over heads - 2 heads per core. Each core computes
q/k/v projections for its 128 output columns, full attention for its 2 heads
(both batches), and a partial out-projection [4096, 1024] in bf16. Host sums
the 8 partials (fp64) and adds the output bias.

Design: one global software pipeline over 128 "slots" (8 groups x 16 key
tiles), emitted in PE program order as slot-PAIRS (adjacent scores pairs
halve the 64x128 <-> 128x128 PE tiling-mode switches). Each slot:
  1. scores pair for (group, kt): both heads as CONCURRENT 64x128 row-tiled
     matmuls (contract = head dim = 64; head h lives on SBUF partitions
     64h..64h+63 in the transposed Q^T/K^T layouts; tile_position (0,0) and
     (64,0), separate psum banks),
  2. one ACT exp over both heads' scores psum [128, 2, 512] (1024
     elems/partition amortizes ACT's ~352-cycle fixed cost; 16.8M exps
     ~= 135us of ACT busy),
  3. the attnV pair trailing by a per-group slot count (16/12/8/4.../2:
     large early trails keep attnV out of the projection-backlog window,
     the last group trails minimally to shrink the tail). V carries an
     all-ones column so psum row 64 accumulates the softmax denominator,
  4. scheduled work at explicit slot offsets: attnV-psum evacuation
     (immediately frees the accumulator banks - psum ring slot reuse must
     never wait on the slow normalization chain), norm_a (+1:
     reciprocal_approx_fast on the [1, 2, 512] denominators - the v1
     per-chunk [1,512] nc.vector.reciprocal was an 8 cyc/elem iterative
     divide that burned 53us of DVE and stalled the PE into HAM clock
     re-throttles), norm_b (+4: partition-broadcast of 1/den via two PE
     rank-1 matmuls - much shorter chain than gpsimd partition_broadcast -
     then two DVE muls into a PER-GROUP ot tile; a shared ot tile makes
     out-proj matmuls false-depend on later norm writes), out-proj row
     tiles (+6.., after the norm chain has actually finished - earlier
     emission stalls the in-order PE stream), with ~6 out-proj units held
     back to the very end to keep the PE busy/warm under the last group's
     norm chain,
  5. one "filler" unit popped from a queue: QK-projection half-blocks and
     V-projection key tiles for the next batch, ordered to respect
     just-in-time dependencies.

PSUM budget (8 banks): scores pairs 2x2 ('sp', double-buffered) + a 4-deep
1-bank ring ('op') shared by attnV accumulators, V/QK projection tiles,
out-proj tiles, and the rank-1 broadcast outputs.
"""

import os
import numpy as np
import ml_dtypes

B, S, D, H = 2, 2048, 1024, 16
HD = D // H          # 64
BS = B * S           # 4096 tokens
NCORES = 8
HPC = H // NCORES    # heads per core = 2
CPC = HPC * HD       # output cols per core = 128
KC = D // 128        # contract chunks = 8
NKT = S // 128       # 16 key tiles per batch
QG = 512             # q-group width (one psum bank of fp32)
NQG = S // QG        # 4 q-groups per batch
# attnV trail per group is set by TRAILS in the slot loop below

BF16 = ml_dtypes.bfloat16

_prog = None


def _build_program():
    import concourse.bacc as bacc
    import concourse.tile as tile
    from concourse import mybir

    f32 = mybir.dt.float32
    bf16 = mybir.dt.bfloat16
    AF = mybir.ActivationFunctionType

    nc = bacc.Bacc("TRN2", debug=False, enable_asserts=False, num_devices=NCORES)

    xT = nc.dram_tensor("xT", [D, BS], bf16, kind="ExternalInput").ap()
    wq = nc.dram_tensor("wq", [D, CPC], bf16, kind="ExternalInput").ap()
    wk = nc.dram_tensor("wk", [D, CPC], bf16, kind="ExternalInput").ap()
    wv = nc.dram_tensor("wv", [D, CPC], bf16, kind="ExternalInput").ap()
    wo = nc.dram_tensor("wo", [CPC, D], bf16, kind="ExternalInput").ap()
    bq = nc.dram_tensor("bq", [CPC, 1], f32, kind="ExternalInput").ap()
    bk = nc.dram_tensor("bk", [CPC, 1], f32, kind="ExternalInput").ap()
    bv = nc.dram_tensor("bv", [1, CPC], bf16, kind="ExternalInput").ap()
    out = nc.dram_tensor("out", [BS, D], bf16, kind="ExternalOutput").ap()

    SCALE = float(1.0 / np.sqrt(HD))

    with tile.TileContext(nc) as tc:
        with (
            tc.tile_pool(name="big", bufs=1) as big,
            tc.tile_pool(name="sm", bufs=1) as sm,
            tc.tile_pool(name="attn", bufs=2) as attn,
            tc.tile_pool(name="etp", bufs=24) as etp,
            tc.tile_pool(name="nrm", bufs=2) as nrm,
            tc.tile_pool(name="ostage", bufs=3) as ostage,
            tc.tile_pool(name="ps", bufs=2, space="PSUM") as ps,
        ):
            # ---- resident SBUF tensors ----
            xt_sb = big.tile([128, KC, BS], bf16, name="xt_sb", tag="xt")
            qt_sb = big.tile([128, BS], bf16, name="qt_sb", tag="qt")
            kt_sb = big.tile([128, BS], bf16, name="kt_sb", tag="kt")
            # V|ones per head: [keys(128) x keytile(32) x (64 V + 1 ones)*2]
            v_sb = big.tile([128, B * NKT, 2 * (HD + 1)], bf16, name="v_sb", tag="v")
            wo_sb = big.tile([128, D], bf16, name="wo_sb", tag="wo")

            wq_sb = sm.tile([128, KC, CPC], bf16, name="wq_sb", tag="wq")
            wk_sb = sm.tile([128, KC, CPC], bf16, name="wk_sb", tag="wk")
            wv_sb = sm.tile([128, KC, CPC], bf16, name="wv_sb", tag="wv")
            bq_sb = sm.tile([CPC, 1], f32, name="bq_sb", tag="bq")
            bk_sb = sm.tile([CPC, 1], f32, name="bk_sb", tag="bk")
            bv_sb = sm.tile([1, CPC], bf16, name="bv_sb", tag="bv")
            ones_bf = sm.tile([1, 128], bf16, name="ones_bf", tag="onesb")

            nc.vector.memset(ones_bf, 1.0)
            nc.vector.memset(v_sb[:, :, HD : HD + 1], 1.0)
            nc.vector.memset(v_sb[:, :, 2 * HD + 1 : 2 * HD + 2], 1.0)

            # ~4.5us of throwaway matmuls during the initial DMA wait: the
            # HAM clock gate starts at K=4/8 (1.2 GHz) and needs ~3.4us of
            # sustained PE activity to unthrottle - warm it up for free so
            # the real lead-in projections run at 2.4 GHz
            scr = sm.tile([1, 512], bf16, name="scr", tag="scr")
            nc.vector.memset(scr, 1.0)
            warm = ps.tile([128, 512], f32, name="warm", tag="sp")
            for _ in range(14):
                nc.tensor.matmul(warm, lhsT=ones_bf, rhs=scr, start=True, stop=True)

            # DMA order: first QK-proj unit needs wq/wk + tokens 0:512 only.
            # The lead-in load alternates the two HWDGE queues (sync + scalar;
            # the ACT engine is idle until the first exp) to halve its latency.
            xt_r = xT.rearrange("(c p) n -> p c n", p=128)
            nc.sync.dma_start(out=wq_sb, in_=wq.rearrange("(c p) n -> p c n", p=128))
            nc.scalar.dma_start(out=wk_sb, in_=wk.rearrange("(c p) n -> p c n", p=128))
            for c in range(KC):
                eng = nc.sync if c % 2 == 0 else nc.scalar
                eng.dma_start(out=xt_sb[:, c, 0:512], in_=xt_r[:, c, 0:512])
            nc.sync.dma_start(out=bq_sb, in_=bq)
            nc.sync.dma_start(out=bk_sb, in_=bk)
            for c in range(KC):
                eng = nc.sync if c % 2 == 0 else nc.scalar
                eng.dma_start(out=xt_sb[:, c, 512:1024], in_=xt_r[:, c, 512:1024])
            for c in range(KC):
                nc.sync.dma_start(out=xt_sb[:, c, 1024:2048], in_=xt_r[:, c, 1024:2048])
            nc.sync.dma_start(out=wv_sb, in_=wv.rearrange("(c p) n -> p c n", p=128))
            nc.sync.dma_start(out=bv_sb, in_=bv)
            for tb in range(2, BS // 1024):
                for c in range(KC):
                    nc.sync.dma_start(
                        out=xt_sb[:, c, tb * 1024 : (tb + 1) * 1024],
                        in_=xt_r[:, c, tb * 1024 : (tb + 1) * 1024],
                    )
            nc.sync.dma_start(out=wo_sb, in_=wo)

            groups = [(b, qg) for b in range(B) for qg in range(NQG)]
            NG = len(groups)

            et_tiles = {}   # (gi, kt) -> et tile [128, 2, QG]
            op_tiles = {}   # (gi, h) -> attnV psum tile [65, QG]
            ot_tiles = {}   # b -> ot_sb [128, S]

            # ---- filler units (each ~1-2us of PE work) ----
            def qk_unit(is_q, half):
                # projects Q^T or K^T for tokens [512*half, 512*(half+1))
                w_sb, b_sb, dst = (
                    (wq_sb, bq_sb, qt_sb) if is_q else (wk_sb, bk_sb, kt_sb)
                )
                tok = half * 512

                def run():
                    pp = ps.tile(
                        [128, 512], f32, name=f"pp{int(is_q)}_{half}", tag="op", bufs=4
                    )
                    for c in range(KC):
                        nc.tensor.matmul(
                            pp,
                            lhsT=w_sb[:, c, :],
                            rhs=xt_sb[:, c, tok : tok + 512],
                            start=(c == 0),
                            stop=(c == KC - 1),
                        )
                    nc.vector.tensor_scalar_add(dst[:, tok : tok + 512], pp, b_sb)

                return run

            def v_unit(kt):
                # V projection (natural [keys, cols] layout) for one key
                # tile; bias via rank-1 ones matmul into the accumulation
                def run():
                    vp = ps.tile([128, CPC], f32, name=f"vp{kt}", tag="op", bufs=4)
                    for c in range(KC):
                        nc.tensor.matmul(
                            vp,
                            lhsT=xt_sb[:, c, kt * 128 : (kt + 1) * 128],
                            rhs=wv_sb[:, c, :],
                            start=(c == 0),
                            stop=False,
                        )
                    nc.tensor.matmul(
                        vp, lhsT=ones_bf, rhs=bv_sb, start=False, stop=True
                    )
                    nc.vector.tensor_copy(
                        v_sb[:, kt, :].rearrange("p (h c) -> p h c", h=2)[:, :, 0:HD],
                        vp.rearrange("p (h c) -> p h c", h=2),
                    )

                return run

            stag_tiles = {}  # gi -> [64, 2, QG] bf16 unnormalized o
            den_tiles = {}   # gi -> [1, 2, QG] f32 softmax denominators

            def evac_group(gi):
                # free the attnV psum accumulators ASAP: pull the denominator
                # row (f32) and the unnormalized output (bf16) into SBUF so
                # the next groups' accumulators never wait on the (slow)
                # normalization chain. For the last group, the exps are done:
                # split the copies across DVE and the now-idle ACT engine so
                # the tail chain (den -> recip -> ...) shortens.
                last = gi == NG - 1
                den = nrm.tile([1, 2, QG], f32, name=f"den{gi}", tag="den")
                stag = nrm.tile([HD, 2, QG], bf16, name=f"stag{gi}", tag="stag")
                for h in range(HPC):
                    op = op_tiles.pop((gi, h))
                    if last and h == 1:
                        nc.scalar.copy(den[0:1, h, :], op[HD : HD + 1, :])
                    else:
                        nc.vector.tensor_copy(den[0:1, h, :], op[HD : HD + 1, :])
                    if last:
                        nc.scalar.copy(stag[:, h, :], op[0:HD, :])
                    else:
                        nc.vector.tensor_copy(stag[:, h, :], op[0:HD, :])
                den_tiles[gi] = den
                stag_tiles[gi] = stag

            rqb_tiles = {}

            def norm_a(gi):
                # DVE half of the normalization: invert the denominators
                def run():
                    den = den_tiles.pop(gi)
                    rq = nrm.tile([1, 2, QG], f32, name=f"rq{gi}", tag="rq")
                    nc.vector.reciprocal_approx_fast(rq, den)
                    rqb = nrm.tile([1, 2, QG], bf16, name=f"rqb{gi}", tag="rqb")
                    nc.vector.tensor_copy(rqb, rq)
                    rqb_tiles[gi] = rqb

                return run

            def norm_b(gi):
                # broadcast 1/den along partitions with two PE rank-1
                # matmuls (ones^T x rqb -> psum) - much shorter chain than a
                # gpsimd partition_broadcast - then scale the staged
                # unnormalized output into the per-group ot tile
                def run():
                    # per-group ot tile: a shared per-batch tile makes the
                    # out-proj matmuls of earlier groups false-depend on this
                    # group's normalization writes (coarse dep tracking)
                    ot_tiles[gi] = attn.tile(
                        [128, QG], bf16, name=f"ot{gi}", tag="ot", bufs=4
                    )
                    ot = ot_tiles[gi]
                    stag = stag_tiles.pop(gi)
                    rqb = rqb_tiles.pop(gi)
                    rbs = {}
                    for h in range(HPC):
                        rbs[h] = ps.tile(
                            [HD, QG], f32, name=f"rbs{gi}_{h}", tag="op", bufs=4
                        )
                        nc.tensor.matmul(
                            rbs[h],
                            lhsT=ones_bf[0:1, 0:HD],
                            rhs=rqb[0:1, h, :],
                            start=True,
                            stop=True,
                        )
                    for h in range(HPC):
                        nc.vector.tensor_mul(
                            ot[h * HD : (h + 1) * HD, :],
                            stag[:, h, :],
                            rbs[h],
                        )

                return run

            def oproj_unit(gi, qt, scalar_evac=False):
                b, qg = groups[gi]

                def run():
                    ot = ot_tiles[gi]
                    lq = qt - qg * (QG // 128)
                    os_ = ostage.tile([128, 1024], bf16, name=f"os{gi}_{qt}", tag="os")
                    for nh in range(2):
                        pq = ps.tile(
                            [128, 512], f32, name=f"pq{gi}{qt}{nh}", tag="op", bufs=4
                        )
                        nc.tensor.matmul(
                            pq,
                            lhsT=ot[:, lq * 128 : (lq + 1) * 128],
                            rhs=wo_sb[:, nh * 512 : (nh + 1) * 512],
                            start=True,
                            stop=True,
                        )
                        # tail units evacuate on the idle ACT engine so the
                        # DVE queue drains the norm chain faster
                        if scalar_evac and nh == 1:
                            nc.scalar.copy(os_[:, nh * 512 : (nh + 1) * 512], pq)
                        else:
                            nc.vector.tensor_copy(os_[:, nh * 512 : (nh + 1) * 512], pq)
                    deng = nc.scalar if scalar_evac and qt % 2 else nc.sync
                    deng.dma_start(
                        out=out[b * S + qt * 128 : b * S + (qt + 1) * 128, :],
                        in_=os_,
                    )

                return run

            # ---- pipeline stages ----
            def scores_pair(gi, kt):
                b, qg = groups[gi]
                q0 = b * S + qg * QG
                k0 = b * S + kt * 128
                sp = ps.tile([128, 2, QG], f32, name=f"sp{gi}_{kt}", tag="sp")
                for h in range(HPC):
                    hp = h * HD
                    nc.tensor.matmul(
                        sp[:, h, :],
                        lhsT=kt_sb[hp : hp + HD, k0 : k0 + 128],
                        rhs=qt_sb[hp : hp + HD, q0 : q0 + QG],
                        start=True,
                        stop=True,
                        tile_position=(hp, 0),
                    )
                et = etp.tile([128, 2, QG], bf16, name=f"et{gi}_{kt}", tag="et")
                nc.scalar.activation(et, sp, AF.Exp, scale=SCALE)
                et_tiles[(gi, kt)] = et

            def attnv_pair(gi, kt):
                b, qg = groups[gi]
                et = et_tiles.pop((gi, kt))
                for h in range(HPC):
                    if kt == 0:
                        op_tiles[(gi, h)] = ps.tile(
                            [HD + 1, QG], f32, name=f"op{gi}_{h}", tag="op", bufs=4
                        )
                    nc.tensor.matmul(
                        op_tiles[(gi, h)],
                        lhsT=v_sb[:, b * NKT + kt, h * (HD + 1) : (h + 1) * (HD + 1)],
                        rhs=et[:, h, :],
                        start=(kt == 0),
                        stop=(kt == NKT - 1),
                    )

            # ---- the slot machine ----
            from collections import deque

            fillers = deque()
            # b0 K halves 1-3 (keys kt4-15; half h covers key tiles 4h..4h+3)
            fillers.append(qk_unit(False, 1))
            fillers.append(qk_unit(False, 2))
            fillers.append(qk_unit(False, 3))
            for kt in range(NKT):             # b0 V (attnV g0 from slot 4)
                fillers.append(v_unit(kt))
            fillers.append(qk_unit(True, 1))  # q for g1 (slot 16)
            fillers.append(qk_unit(True, 2))  # q for g2 (slot 32)
            fillers.append(qk_unit(True, 3))  # q for g3 (slot 48)
            for half in range(4, 8):          # b1 keys (slot 64)
                fillers.append(qk_unit(False, half))
            fillers.append(qk_unit(True, 4))  # q for g4 (slot 64)
            for kt in range(NKT, 2 * NKT):    # b1 V (attnV g4 from slot 68)
                fillers.append(v_unit(kt))
            fillers.append(qk_unit(True, 5))
            fillers.append(qk_unit(True, 6))
            fillers.append(qk_unit(True, 7))

            # lead-in: q/k needed by group 0's first scores (key tiles 0-3)
            qk_unit(True, 0)()
            qk_unit(False, 0)()

            import heapq

            scheduled = []  # heap of (target_slot, seq, fn)
            seq_ctr = [0]

            def schedule(target, fn):
                heapq.heappush(scheduled, (target, seq_ctr[0], fn))
                seq_ctr[0] += 1

            def make_attnv(gi, kt):
                def run_at(slot):
                    attnv_pair(gi, kt)
                    if kt == NKT - 1:
                        evac_group(gi)
                        b, qg = groups[gi]
                        qts = list(range(qg * (QG // 128), (qg + 1) * (QG // 128)))
                        last = gi == NG - 1
                        schedule(slot + 1, lambda s: norm_a(gi)())
                        schedule(slot + (3 if last else 4), lambda s: norm_b(gi)())
                        if gi >= NG - 4 and not last:
                            # hold back ~6 late out-proj row-tiles: they keep
                            # the PE busy (and the HAM clock warm) while the
                            # last group's normalization chain runs
                            keep = {NG - 4: 3, NG - 3: 2, NG - 2: 1}[gi]
                            tail = NG * NKT + 2 + (NG - 1 - gi)
                            for j, qt in enumerate(qts[keep:]):
                                schedule(tail + j, (lambda q: lambda s: oproj_unit(gi, q, scalar_evac=True)())(qt))
                            qts = qts[:keep]
                        # ot(gi) is ready ~5-6 slots after the norm chain
                        # starts; emitting out-proj earlier stalls the
                        # in-order PE stream on the DVE normalization
                        off = 5 if last else 6
                        for j, qt in enumerate(qts):
                            schedule(slot + off + j, (lambda q: lambda s: oproj_unit(gi, q, scalar_evac=last)())(qt))

                return run_at

            def emit_rest(slot):
                ran = 0
                while scheduled and scheduled[0][0] <= slot:
                    _, _, fn = heapq.heappop(scheduled)
                    fn(slot)
                    ran += 1
                npop = 2 if slot < 2 * NKT else 1
                for _ in range(npop):
                    if fillers:
                        fillers.popleft()()

            # attnV trails scores per group: large trails early push attnV's
            # PE work out of the projection-backlog window (ACT would starve
            # otherwise); the last group trails minimally to shrink the tail.
            # Slots run in pairs with both scores pairs adjacent: halves the
            # 64x128 <-> 128x128 PE tiling-mode switches, and since the PE
            # runs behind the ACT-paced scores stream, the second pair's sp
            # ring wait (exp from one slot back) never actually stalls.
            TRAILS = [16, 12, 8, 4, 4, 4, 4, 2]
            for slot in range(0, NG * NKT, 2):
                for s in (slot, slot + 1):
                    gi, kt = divmod(s, NKT)
                    scores_pair(gi, kt)
                    # round the attnV target up to a slot-pair boundary so
                    # consecutive kt attnV pairs emit adjacently (their
                    # LDWEIGHTS pipeline across 4 matmuls)
                    tgt = s + TRAILS[gi]
                    schedule(tgt + (tgt & 1), make_attnv(gi, kt))
                emit_rest(slot)
                emit_rest(slot + 1)
            slot = NG * NKT
            while scheduled or fillers:
                emit_rest(slot)
                slot += 1

    nc.compile()
    return nc


def _get_prog():
    global _prog
    if _prog is None:
        _prog = _build_program()
    return _prog


def kernel(x, Wq, bq, Wk, bk, Wv, bv, Wo, bo):
    from concourse import bass_utils

    nc = _get_prog()

    xT = np.ascontiguousarray(
        np.asarray(x, dtype=np.float32).reshape(BS, D).T
    ).astype(BF16)

    in_maps = []
    for c in range(NCORES):
        cols = slice(c * CPC, (c + 1) * CPC)
        in_maps.append(
            {
                "xT": xT,
                "wq": np.ascontiguousarray(Wq[cols, :].T).astype(BF16),
                "wk": np.ascontiguousarray(Wk[cols, :].T).astype(BF16),
                "wv": np.ascontiguousarray(Wv[cols, :].T).astype(BF16),
                "wo": np.ascontiguousarray(Wo[:, cols].T).astype(BF16),
                "bq": np.asarray(bq[cols], np.float32).reshape(CPC, 1),
                "bk": np.asarray(bk[cols], np.float32).reshape(CPC, 1),
                "bv": np.asarray(bv[cols], np.float32).reshape(1, CPC).astype(BF16),
            }
        )

    res = bass_utils.run_bass_kernel_spmd(
        nc,
        in_maps,
        core_ids=list(range(NCORES)),
        trace=bool(int(os.environ.get("KERNEL_TRACE", "0"))),
    )
    kernel.last_results = res

    acc = np.zeros((BS, D), np.float64)
    for c in range(NCORES):
        acc += res.results[c]["out"].astype(np.float64)
    acc += np.asarray(bo, np.float64)[None, :]
    return acc.reshape(B, S, D).astype(np.float32)


# revision 55
# speedup vs baseline: 1.0405x; 1.0405x over previous
"""Multi-head attention (B=2, S=2048, D=1024, H=16) on 8 NeuronCores.

Sharding: tensor-parallel over heads - 2 heads per core. Each core computes
q/k/v projections for its 128 output columns, full attention for its 2 heads
(both batches), and a partial out-projection [4096, 1024] in bf16. Host sums
the 8 partials (fp64) and adds the output bias.

Design: one global software pipeline over 128 "slots" (8 groups x 16 key
tiles), emitted in PE program order as slot-PAIRS (adjacent scores pairs
halve the 64x128 <-> 128x128 PE tiling-mode switches). Each slot:
  1. scores pair for (group, kt): both heads as CONCURRENT 64x128 row-tiled
     matmuls (contract = head dim = 64; head h lives on SBUF partitions
     64h..64h+63 in the transposed Q^T/K^T layouts; tile_position (0,0) and
     (64,0), separate psum banks),
  2. one ACT exp over both heads' scores psum [128, 2, 512] (1024
     elems/partition amortizes ACT's ~352-cycle fixed cost; 16.8M exps
     ~= 135us of ACT busy),
  3. the attnV pair trailing by a per-group slot count (16/12/8/4.../2:
     large early trails keep attnV out of the projection-backlog window,
     the last group trails minimally to shrink the tail). V carries an
     all-ones column so psum row 64 accumulates the softmax denominator,
  4. scheduled work at explicit slot offsets: attnV-psum evacuation
     (immediately frees the accumulator banks - psum ring slot reuse must
     never wait on the slow normalization chain), norm_a (+1:
     reciprocal_approx_fast on the [1, 2, 512] denominators - the v1
     per-chunk [1,512] nc.vector.reciprocal was an 8 cyc/elem iterative
     divide that burned 53us of DVE and stalled the PE into HAM clock
     re-throttles), norm_b (+4: partition-broadcast of 1/den via two PE
     rank-1 matmuls - much shorter chain than gpsimd partition_broadcast -
     then two DVE muls into a PER-GROUP ot tile; a shared ot tile makes
     out-proj matmuls false-depend on later norm writes), out-proj row
     tiles (+6.., after the norm chain has actually finished - earlier
     emission stalls the in-order PE stream), with ~6 out-proj units held
     back to the very end to keep the PE busy/warm under the last group's
     norm chain,
  5. one "filler" unit popped from a queue: QK-projection half-blocks and
     V-projection key tiles for the next batch, ordered to respect
     just-in-time dependencies.

PSUM budget (8 banks): scores pairs 2x2 ('sp', double-buffered) + a 4-deep
1-bank ring ('op') shared by attnV accumulators, V/QK projection tiles,
out-proj tiles, and the rank-1 broadcast outputs.
"""

import os
import numpy as np
import ml_dtypes

B, S, D, H = 2, 2048, 1024, 16
HD = D // H          # 64
BS = B * S           # 4096 tokens
NCORES = 8
HPC = H // NCORES    # heads per core = 2
CPC = HPC * HD       # output cols per core = 128
KC = D // 128        # contract chunks = 8
NKT = S // 128       # 16 key tiles per batch
QG = 512             # q-group width (one psum bank of fp32)
NQG = S // QG        # 4 q-groups per batch
# attnV trail per group is set by TRAILS in the slot loop below

BF16 = ml_dtypes.bfloat16

_prog = None


def _build_program():
    import concourse.bacc as bacc
    import concourse.tile as tile
    from concourse import mybir

    f32 = mybir.dt.float32
    bf16 = mybir.dt.bfloat16
    AF = mybir.ActivationFunctionType

    nc = bacc.Bacc("TRN2", debug=False, enable_asserts=False, num_devices=NCORES)

    xT = nc.dram_tensor("xT", [D, BS], bf16, kind="ExternalInput").ap()
    wq = nc.dram_tensor("wq", [D, CPC], bf16, kind="ExternalInput").ap()
    wk = nc.dram_tensor("wk", [D, CPC], bf16, kind="ExternalInput").ap()
    wv = nc.dram_tensor("wv", [D, CPC], bf16, kind="ExternalInput").ap()
    wo = nc.dram_tensor("wo", [CPC, D], bf16, kind="ExternalInput").ap()
    bq = nc.dram_tensor("bq", [CPC, 1], f32, kind="ExternalInput").ap()
    bk = nc.dram_tensor("bk", [CPC, 1], f32, kind="ExternalInput").ap()
    bv = nc.dram_tensor("bv", [1, CPC], bf16, kind="ExternalInput").ap()
    out = nc.dram_tensor("out", [BS, D], bf16, kind="ExternalOutput").ap()

    SCALE = float(1.0 / np.sqrt(HD))

    with tile.TileContext(nc) as tc:
        with (
            tc.tile_pool(name="big", bufs=1) as big,
            tc.tile_pool(name="sm", bufs=1) as sm,
            tc.tile_pool(name="attn", bufs=2) as attn,
            tc.tile_pool(name="etp", bufs=24) as etp,
            tc.tile_pool(name="nrm", bufs=2) as nrm,
            tc.tile_pool(name="ostage", bufs=3) as ostage,
            tc.tile_pool(name="ps", bufs=2, space="PSUM") as ps,
        ):
            # ---- resident SBUF tensors ----
            xt_sb = big.tile([128, KC, BS], bf16, name="xt_sb", tag="xt")
            qt_sb = big.tile([128, BS], bf16, name="qt_sb", tag="qt")
            kt_sb = big.tile([128, BS], bf16, name="kt_sb", tag="kt")
            # V|ones per head: [keys(128) x keytile(32) x (64 V + 1 ones)*2]
            v_sb = big.tile([128, B * NKT, 2 * (HD + 1)], bf16, name="v_sb", tag="v")
            wo_sb = big.tile([128, D], bf16, name="wo_sb", tag="wo")

            wq_sb = sm.tile([128, KC, CPC], bf16, name="wq_sb", tag="wq")
            wk_sb = sm.tile([128, KC, CPC], bf16, name="wk_sb", tag="wk")
            wv_sb = sm.tile([128, KC, CPC], bf16, name="wv_sb", tag="wv")
            bq_sb = sm.tile([CPC, 1], f32, name="bq_sb", tag="bq")
            bk_sb = sm.tile([CPC, 1], f32, name="bk_sb", tag="bk")
            bv_sb = sm.tile([1, CPC], bf16, name="bv_sb", tag="bv")
            ones_bf = sm.tile([1, 128], bf16, name="ones_bf", tag="onesb")

            nc.vector.memset(ones_bf, 1.0)
            nc.vector.memset(v_sb[:, :, HD : HD + 1], 1.0)
            nc.vector.memset(v_sb[:, :, 2 * HD + 1 : 2 * HD + 2], 1.0)

            # ~4.5us of throwaway matmuls during the initial DMA wait: the
            # HAM clock gate starts at K=4/8 (1.2 GHz) and needs ~3.4us of
            # sustained PE activity to unthrottle - warm it up for free so
            # the real lead-in projections run at 2.4 GHz
            scr = sm.tile([1, 512], bf16, name="scr", tag="scr")
            nc.vector.memset(scr, 1.0)
            warm = ps.tile([128, 512], f32, name="warm", tag="sp")
            for _ in range(14):
                nc.tensor.matmul(warm, lhsT=ones_bf, rhs=scr, start=True, stop=True)

            # DMA order: first QK-proj unit needs wq/wk + tokens 0:512 only.
            # The lead-in load alternates the two HWDGE queues (sync + scalar;
            # the ACT engine is idle until the first exp) to halve its latency.
            xt_r = xT.rearrange("(c p) n -> p c n", p=128)
            nc.sync.dma_start(out=wq_sb, in_=wq.rearrange("(c p) n -> p c n", p=128))
            nc.scalar.dma_start(out=wk_sb, in_=wk.rearrange("(c p) n -> p c n", p=128))
            for c in range(KC):
                eng = nc.sync if c % 2 == 0 else nc.scalar
                eng.dma_start(out=xt_sb[:, c, 0:512], in_=xt_r[:, c, 0:512])
            nc.sync.dma_start(out=bq_sb, in_=bq)
            nc.sync.dma_start(out=bk_sb, in_=bk)
            for c in range(KC):
                eng = nc.sync if c % 2 == 0 else nc.scalar
                eng.dma_start(out=xt_sb[:, c, 512:1024], in_=xt_r[:, c, 512:1024])
            # wv before the rest of x: V units for key tiles 0-3 only need
            # tokens 0:512 + wv, giving the PE runnable filler work while the
            # remaining x stream is still in flight (otherwise the early
            # backlog stalls on DMA and re-throttles the HAM clock)
            nc.scalar.dma_start(out=wv_sb, in_=wv.rearrange("(c p) n -> p c n", p=128))
            nc.scalar.dma_start(out=bv_sb, in_=bv)
            for c in range(KC):
                nc.sync.dma_start(out=xt_sb[:, c, 1024:2048], in_=xt_r[:, c, 1024:2048])
            for tb in range(2, BS // 1024):
                for c in range(KC):
                    nc.sync.dma_start(
                        out=xt_sb[:, c, tb * 1024 : (tb + 1) * 1024],
                        in_=xt_r[:, c, tb * 1024 : (tb + 1) * 1024],
                    )
            nc.sync.dma_start(out=wo_sb, in_=wo)

            groups = [(b, qg) for b in range(B) for qg in range(NQG)]
            NG = len(groups)

            et_tiles = {}   # (gi, kt) -> et tile [128, 2, QG]
            op_tiles = {}   # (gi, h) -> attnV psum tile [65, QG]
            ot_tiles = {}   # b -> ot_sb [128, S]

            # ---- filler units (each ~1-2us of PE work) ----
            def qk_unit(is_q, half):
                # projects Q^T or K^T for tokens [512*half, 512*(half+1))
                w_sb, b_sb, dst = (
                    (wq_sb, bq_sb, qt_sb) if is_q else (wk_sb, bk_sb, kt_sb)
                )
                tok = half * 512

                def run():
                    pp = ps.tile(
                        [128, 512], f32, name=f"pp{int(is_q)}_{half}", tag="op", bufs=4
                    )
                    for c in range(KC):
                        nc.tensor.matmul(
                            pp,
                            lhsT=w_sb[:, c, :],
                            rhs=xt_sb[:, c, tok : tok + 512],
                            start=(c == 0),
                            stop=(c == KC - 1),
                        )
                    nc.vector.tensor_scalar_add(dst[:, tok : tok + 512], pp, b_sb)

                return run

            def v_unit(kt):
                # V projection (natural [keys, cols] layout) for one key
                # tile; bias via rank-1 ones matmul into the accumulation
                def run():
                    vp = ps.tile([128, CPC], f32, name=f"vp{kt}", tag="op", bufs=4)
                    for c in range(KC):
                        nc.tensor.matmul(
                            vp,
                            lhsT=xt_sb[:, c, kt * 128 : (kt + 1) * 128],
                            rhs=wv_sb[:, c, :],
                            start=(c == 0),
                            stop=False,
                        )
                    nc.tensor.matmul(
                        vp, lhsT=ones_bf, rhs=bv_sb, start=False, stop=True
                    )
                    nc.vector.tensor_copy(
                        v_sb[:, kt, :].rearrange("p (h c) -> p h c", h=2)[:, :, 0:HD],
                        vp.rearrange("p (h c) -> p h c", h=2),
                    )

                return run

            stag_tiles = {}  # gi -> [64, 2, QG] bf16 unnormalized o
            den_tiles = {}   # gi -> [1, 2, QG] f32 softmax denominators

            def evac_group(gi):
                # free the attnV psum accumulators ASAP: pull the denominator
                # row (f32) and the unnormalized output (bf16) into SBUF so
                # the next groups' accumulators never wait on the (slow)
                # normalization chain. For the last group, the exps are done:
                # split the copies across DVE and the now-idle ACT engine so
                # the tail chain (den -> recip -> ...) shortens.
                last = gi == NG - 1
                den = nrm.tile([1, 2, QG], f32, name=f"den{gi}", tag="den")
                stag = nrm.tile([HD, 2, QG], bf16, name=f"stag{gi}", tag="stag")
                for h in range(HPC):
                    op = op_tiles.pop((gi, h))
                    if last and h == 1:
                        nc.scalar.copy(den[0:1, h, :], op[HD : HD + 1, :])
                    else:
                        nc.vector.tensor_copy(den[0:1, h, :], op[HD : HD + 1, :])
                    if last:
                        nc.scalar.copy(stag[:, h, :], op[0:HD, :])
                    else:
                        nc.vector.tensor_copy(stag[:, h, :], op[0:HD, :])
                den_tiles[gi] = den
                stag_tiles[gi] = stag

            rqb_tiles = {}

            def norm_a(gi):
                # DVE half of the normalization: invert the denominators
                def run():
                    den = den_tiles.pop(gi)
                    rq = nrm.tile([1, 2, QG], f32, name=f"rq{gi}", tag="rq")
                    nc.vector.reciprocal_approx_fast(rq, den)
                    rqb = nrm.tile([1, 2, QG], bf16, name=f"rqb{gi}", tag="rqb")
                    nc.vector.tensor_copy(rqb, rq)
                    rqb_tiles[gi] = rqb

                return run

            def norm_b(gi):
                # broadcast 1/den along partitions with two PE rank-1
                # matmuls (ones^T x rqb -> psum) - much shorter chain than a
                # gpsimd partition_broadcast - then scale the staged
                # unnormalized output into the per-group ot tile
                def run():
                    # per-group ot tile: a shared per-batch tile makes the
                    # out-proj matmuls of earlier groups false-depend on this
                    # group's normalization writes (coarse dep tracking)
                    ot_tiles[gi] = attn.tile(
                        [128, QG], bf16, name=f"ot{gi}", tag="ot", bufs=4
                    )
                    ot = ot_tiles[gi]
                    stag = stag_tiles.pop(gi)
                    rqb = rqb_tiles.pop(gi)
                    rbs = {}
                    for h in range(HPC):
                        rbs[h] = ps.tile(
                            [HD, QG], f32, name=f"rbs{gi}_{h}", tag="op", bufs=4
                        )
                        nc.tensor.matmul(
                            rbs[h],
                            lhsT=ones_bf[0:1, 0:HD],
                            rhs=rqb[0:1, h, :],
                            start=True,
                            stop=True,
                        )
                    for h in range(HPC):
                        nc.vector.tensor_mul(
                            ot[h * HD : (h + 1) * HD, :],
                            stag[:, h, :],
                            rbs[h],
                        )

                return run

            def oproj_unit(gi, qt, scalar_evac=False):
                b, qg = groups[gi]

                def run():
                    ot = ot_tiles[gi]
                    lq = qt - qg * (QG // 128)
                    os_ = ostage.tile([128, 1024], bf16, name=f"os{gi}_{qt}", tag="os")
                    for nh in range(2):
                        pq = ps.tile(
                            [128, 512], f32, name=f"pq{gi}{qt}{nh}", tag="op", bufs=4
                        )
                        nc.tensor.matmul(
                            pq,
                            lhsT=ot[:, lq * 128 : (lq + 1) * 128],
                            rhs=wo_sb[:, nh * 512 : (nh + 1) * 512],
                            start=True,
                            stop=True,
                        )
                        # tail units evacuate on the idle ACT engine so the
                        # DVE queue drains the norm chain faster
                        if scalar_evac and nh == 1:
                            nc.scalar.copy(os_[:, nh * 512 : (nh + 1) * 512], pq)
                        else:
                            nc.vector.tensor_copy(os_[:, nh * 512 : (nh + 1) * 512], pq)
                    deng = nc.scalar if scalar_evac and qt % 2 else nc.sync
                    deng.dma_start(
                        out=out[b * S + qt * 128 : b * S + (qt + 1) * 128, :],
                        in_=os_,
                    )

                return run

            # ---- pipeline stages ----
            def scores_pair(gi, kt):
                b, qg = groups[gi]
                q0 = b * S + qg * QG
                k0 = b * S + kt * 128
                sp = ps.tile([128, 2, QG], f32, name=f"sp{gi}_{kt}", tag="sp")
                for h in range(HPC):
                    hp = h * HD
                    nc.tensor.matmul(
                        sp[:, h, :],
                        lhsT=kt_sb[hp : hp + HD, k0 : k0 + 128],
                        rhs=qt_sb[hp : hp + HD, q0 : q0 + QG],
                        start=True,
                        stop=True,
                        tile_position=(hp, 0),
                    )
                et = etp.tile([128, 2, QG], bf16, name=f"et{gi}_{kt}", tag="et")
                nc.scalar.activation(et, sp, AF.Exp, scale=SCALE)
                et_tiles[(gi, kt)] = et

            def attnv_pair(gi, kt):
                b, qg = groups[gi]
                et = et_tiles.pop((gi, kt))
                for h in range(HPC):
                    if kt == 0:
                        op_tiles[(gi, h)] = ps.tile(
                            [HD + 1, QG], f32, name=f"op{gi}_{h}", tag="op", bufs=4
                        )
                    nc.tensor.matmul(
                        op_tiles[(gi, h)],
                        lhsT=v_sb[:, b * NKT + kt, h * (HD + 1) : (h + 1) * (HD + 1)],
                        rhs=et[:, h, :],
                        start=(kt == 0),
                        stop=(kt == NKT - 1),
                    )

            # ---- the slot machine ----
            from collections import deque

            fillers = deque()
            # ordered so early pops only touch data the DMA stream has
            # already delivered (x tokens 0:1024 + wv first)
            fillers.append(qk_unit(False, 1))   # keys kt4-7 (x 512:1024)
            for kt in range(4):                 # V kt0-3 (x 0:512 + wv)
                fillers.append(v_unit(kt))
            fillers.append(qk_unit(False, 2))   # keys kt8-11 (x 1024:1536)
            fillers.append(qk_unit(False, 3))   # keys kt12-15
            for kt in range(4, 8):              # V kt4-7
                fillers.append(v_unit(kt))
            fillers.append(qk_unit(True, 1))    # q for g1 (slot 16)
            for kt in range(8, NKT):            # V kt8-15
                fillers.append(v_unit(kt))
            fillers.append(qk_unit(True, 2))    # q for g2 (slot 32)
            fillers.append(qk_unit(True, 3))    # q for g3 (slot 48)
            for kt in range(NKT, NKT + 8):      # b1 V (attnV g4 from slot 68)
                fillers.append(v_unit(kt))
            for half in range(4, 8):            # b1 keys (slot 64)
                fillers.append(qk_unit(False, half))
            fillers.append(qk_unit(True, 4))    # q for g4 (slot 64)
            for kt in range(NKT + 8, 2 * NKT):
                fillers.append(v_unit(kt))
            fillers.append(qk_unit(True, 5))
            fillers.append(qk_unit(True, 6))
            fillers.append(qk_unit(True, 7))

            # lead-in: q/k needed by group 0's first scores (key tiles 0-3)
            qk_unit(True, 0)()
            qk_unit(False, 0)()

            import heapq

            scheduled = []  # heap of (target_slot, seq, fn)
            seq_ctr = [0]

            def schedule(target, fn):
                heapq.heappush(scheduled, (target, seq_ctr[0], fn))
                seq_ctr[0] += 1

            def make_attnv(gi, kt):
                def run_at(slot):
                    attnv_pair(gi, kt)
                    if kt == NKT - 1:
                        evac_group(gi)
                        b, qg = groups[gi]
                        qts = list(range(qg * (QG // 128), (qg + 1) * (QG // 128)))
                        last = gi == NG - 1
                        schedule(slot + 1, lambda s: norm_a(gi)())
                        schedule(slot + (3 if last else 4), lambda s: norm_b(gi)())
                        if gi >= NG - 4 and not last:
                            # hold back ~6 late out-proj row-tiles: they keep
                            # the PE busy (and the HAM clock warm) while the
                            # last group's normalization chain runs
                            keep = {NG - 4: 3, NG - 3: 2, NG - 2: 1}[gi]
                            tail = NG * NKT + 2 + (NG - 1 - gi)
                            for j, qt in enumerate(qts[keep:]):
                                schedule(tail + j, (lambda q: lambda s: oproj_unit(gi, q, scalar_evac=True)())(qt))
                            qts = qts[:keep]
                        # ot(gi) is ready ~5-6 slots after the norm chain
                        # starts; emitting out-proj earlier stalls the
                        # in-order PE stream on the DVE normalization
                        off = 5 if last else 6
                        for j, qt in enumerate(qts):
                            schedule(slot + off + j, (lambda q: lambda s: oproj_unit(gi, q, scalar_evac=last)())(qt))

                return run_at

            def emit_rest(slot):
                ran = 0
                while scheduled and scheduled[0][0] <= slot:
                    _, _, fn = heapq.heappop(scheduled)
                    fn(slot)
                    ran += 1
                # drain the backlog fast only while its data is resident;
                # popping 2/slot into the batch-1 DMA window stalls on DMA
                npop = 2 if slot < 12 else 1
                for _ in range(npop):
                    if fillers:
                        fillers.popleft()()

            # attnV trails scores per group: large trails early push attnV's
            # PE work out of the projection-backlog window (ACT would starve
            # otherwise); the last group trails minimally to shrink the tail.
            # Slots run in pairs with both scores pairs adjacent: halves the
            # 64x128 <-> 128x128 PE tiling-mode switches, and since the PE
            # runs behind the ACT-paced scores stream, the second pair's sp
            # ring wait (exp from one slot back) never actually stalls.
            TRAILS = [16, 12, 8, 4, 4, 4, 4, 2]
            for slot in range(0, NG * NKT, 2):
                for s in (slot, slot + 1):
                    gi, kt = divmod(s, NKT)
                    scores_pair(gi, kt)
                    # round the attnV target up to a slot-pair boundary so
                    # consecutive kt attnV pairs emit adjacently (their
                    # LDWEIGHTS pipeline across 4 matmuls)
                    tgt = s + TRAILS[gi]
                    schedule(tgt + (tgt & 1), make_attnv(gi, kt))
                emit_rest(slot)
                emit_rest(slot + 1)
            slot = NG * NKT
            while scheduled or fillers:
                emit_rest(slot)
                slot += 1

    nc.compile()
    return nc


def _get_prog():
    global _prog
    if _prog is None:
        _prog = _build_program()
    return _prog


def kernel(x, Wq, bq, Wk, bk, Wv, bv, Wo, bo):
    from concourse import bass_utils

    nc = _get_prog()

    xT = np.ascontiguousarray(
        np.asarray(x, dtype=np.float32).reshape(BS, D).T
    ).astype(BF16)

    in_maps = []
    for c in range(NCORES):
        cols = slice(c * CPC, (c + 1) * CPC)
        in_maps.append(
            {
                "xT": xT,
                "wq": np.ascontiguousarray(Wq[cols, :].T).astype(BF16),
                "wk": np.ascontiguousarray(Wk[cols, :].T).astype(BF16),
                "wv": np.ascontiguousarray(Wv[cols, :].T).astype(BF16),
                "wo": np.ascontiguousarray(Wo[:, cols].T).astype(BF16),
                "bq": np.asarray(bq[cols], np.float32).reshape(CPC, 1),
                "bk": np.asarray(bk[cols], np.float32).reshape(CPC, 1),
                "bv": np.asarray(bv[cols], np.float32).reshape(1, CPC).astype(BF16),
            }
        )

    res = bass_utils.run_bass_kernel_spmd(
        nc,
        in_maps,
        core_ids=list(range(NCORES)),
        trace=bool(int(os.environ.get("KERNEL_TRACE", "0"))),
    )
    kernel.last_results = res

    acc = np.zeros((BS, D), np.float64)
    for c in range(NCORES):
        acc += res.results[c]["out"].astype(np.float64)
    acc += np.asarray(bo, np.float64)[None, :]
    return acc.reshape(B, S, D).astype(np.float32)
